# revision 46
# baseline (speedup 1.0000x reference)
"""EulerCTRNN Bass/Tile kernel for Trainium2, data-parallel over batch on 8 cores.

Math (per step t):
    h'      = 0.9*h + tanh(h) @ (0.1*Wh) + U_t
    U_t     = 0.1*(x_t @ Wi) + 0.1*bh + 0.01*noise_t   (precomputed, SBUF-resident)
    rates_t = tanh(h')
    z_t     = rates_t @ Wo + bo        (accumulated in a PSUM bank via tiny MMs)

On-chip layout is transposed: h, tanh(h), U live as [128 partitions = h%128,
cols] with cols m*BL+b (m = h//128). The recurrent matmul runs with Wh' tiles
stationary and the small batch dim moving:
    drive_T[m] = sum_k Wh'[k-tile, m-tile].T @ tanh_T[k-tile]

Host does pure data marshalling: slicing the batch across cores, transposing
x/noise to [feat, b, t] (t-contiguous) for DMA-friendly loads, and transposing
the [H, BL, T] rates output back to [B, T, H].
"""

import numpy as np

try:
    import concourse.bass as bass
except ImportError:  # concourse lives in /opt/trn_rl_repo on these images
    import sys
    sys.path.insert(0, "/opt/trn_rl_repo")
    import concourse.bass as bass

import concourse.mybir as mybir
from concourse.tile import TileContext
from concourse.vector_clock import ScopedClock
from concourse.bass_utils import run_bass_kernel_spmd

F32 = mybir.dt.float32
BF16 = mybir.dt.bfloat16
F16 = mybir.dt.float16
MULT = mybir.AluOpType.mult
ADD = mybir.AluOpType.add
TANH = mybir.ActivationFunctionType.Tanh
COPY = mybir.ActivationFunctionType.Copy

# Problem constants (hardcoded per contest rules).
B, T, I, H = 128, 512, 128, 512
NCORES = 8
BL = B // NCORES          # 16 batch rows per core
KT = H // 128             # 4 contraction tiles
MT = H // 128             # 4 output tiles
W = MT * BL               # 64 = cols of one step tile
SG = 64                   # steps per rates-staging DMA group (=> 256B runs)
ALPHA = 0.1
NOISE = 0.1

# Wh matmul operand dtype: bf16 halves the LDWEIGHTS stream via FWL.
MM_DTYPE = "bf16"


def _install_drain_patch():
    """walrus here rejects >1 sync-wait on a CTRL (Drain) instruction; Tile's
    tail drain aggregates one wait per logical processor onto one Drain.
    Split into one Drain per wait (same engine, program order => equivalent)."""
    if getattr(TileContext, "_ant_drain_patch", False):
        return

    def _patched(self, tick_clock, wait_clock):
        nc = self.nc
        drain_inst = nc.sync.drain()
        wait_clock.add_sem_waits(
            drain_inst.ins, ScopedClock({None: tick_clock.global_clock})
        )
        si = drain_inst.ins.sync_info
        waits = list(si.on_wait) if si is not None else []
        if len(waits) > 1:
            si.on_wait.clear()
            si.on_wait.append(waits[0])
            for w in waits[1:]:
                extra = nc.sync.drain()
                extra.ins.sync_info = mybir.SyncInfo(on_wait=[w], on_update=[])
        nc.all_engine_barrier()
        assert self.sems is not None
        popped = nc._tile_sem_poison_stack.pop()
        assert popped is self._sem_poison
        nc.clear_and_free_semaphores(list(self.sems.allocated().values()))
        nc.all_engine_barrier()

    TileContext._drain_and_barrier = _patched
    TileContext._ant_drain_patch = True


_MAX_WAITS = 1  # this walrus build rejects >1 sync-wait per instruction


def _install_ldw_opt_patch():
    """Re-enable walrus LDWEIGHTS elision (repeated identical stationaries,
    e.g. the z-pass wo reloads). bass_utils hardcodes =false; intercept the
    walrus command line."""
    return  # =true crashes this walrus build (visitInstLdweights); keep off


def _split_excess_waits(nc):
    """Hoist overflow sync-waits onto standalone EventSemaphore instructions
    inserted just before the offending instruction on the same engine."""
    seq = 0
    for f in nc.m.functions:
        for blk in f.blocks:
            insts = blk.instructions
            i = 0
            while i < len(insts):
                inst = insts[i]
                si = getattr(inst, "sync_info", None)
                waits = list(si.on_wait) if si is not None and si.on_wait else []
                if len(waits) > _MAX_WAITS:
                    keep = waits[: _MAX_WAITS]
                    overflow = waits[_MAX_WAITS :]
                    si.on_wait.clear()
                    for w in keep:
                        si.on_wait.append(w)
                    pos = i
                    for j in range(0, len(overflow), _MAX_WAITS):
                        seq += 1
                        ev = mybir.InstEventSemaphore(
                            name=f"antwaitsplit-{seq}",
                            engine=inst.engine,
                            ins=[],
                            outs=[],
                            sync_info=mybir.SyncInfo(
                                on_wait=list(overflow[j : j + _MAX_WAITS]),
                                on_update=[],
                            ),
                        )
                        insts.insert(pos, ev)
                        pos += 1
                        i += 1
                i += 1


def build_nc(bo_val: float, mm_dtype: str = MM_DTYPE, t_steps: int = T):
    _install_drain_patch()
    _install_ldw_opt_patch()
    mmdt = BF16 if mm_dtype == "bf16" else F32
    nc = bass.Bass()

    # Host-marshalled inputs: xl_t = x[core].T -> [I, BL, T]; nl_t likewise.
    xl_t = nc.dram_tensor("xl_t", [I, BL, T], mmdt, kind="ExternalInput")
    nl_t = nc.dram_tensor("nl_t", [H, BL, T], F32, kind="ExternalInput")
    whp = nc.dram_tensor("whp", [H, H], mmdt, kind="ExternalInput")
    wi = nc.dram_tensor("wi", [I, H], mmdt, kind="ExternalInput")
    bhp = nc.dram_tensor("bhp", [H], F32, kind="ExternalInput")
    wo = nc.dram_tensor("wo", [H], mmdt, kind="ExternalInput")
    ident = nc.dram_tensor("ident", [128, 128], F16, kind="ExternalInput")
    rates_t = nc.dram_tensor("rates_t", [H, BL, T], mmdt, kind="ExternalOutput")
    # z in grouped layout: [g, hb, b, s] = z[hb*8+b, g*SG+s]; host decodes
    z_o = nc.dram_tensor("z", [(T // SG) * BL * SG], F32, kind="ExternalOutput")
    assert t_steps % SG == 0, "z group pass assumes full SG groups"

    with TileContext(nc) as tc:
        with (
            tc.tile_pool(name="const", bufs=1) as cpool,
            tc.tile_pool(name="ubig", bufs=1) as upool,
        ):
            # ---- weights / state ----
            wh_sb = cpool.tile([128, KT * MT * 128], mmdt, name="wh_sb")
            for k in range(KT):
                # wh_sb col (k*MT+m)*128+j = Wh'[k*128+p, m*128+j]
                nc.sync.dma_start(
                    wh_sb[:, k * 512 : (k + 1) * 512],
                    whp[k * 128 : (k + 1) * 128, :],
                )
            wi_sb = cpool.tile([128, MT * 128], mmdt, name="wi_sb")
            nc.sync.dma_start(wi_sb[:, :], wi[:, :])
            bh_sb = cpool.tile([128, MT], F32, name="bh_sb")
            wo_sb = cpool.tile([128, MT], mmdt, name="wo_sb")
            for m in range(MT):
                nc.sync.dma_start(
                    bh_sb[:, m : m + 1], bhp.rearrange("(m p) -> p m", p=128)[:, m : m + 1]
                )
                nc.sync.dma_start(
                    wo_sb[:, m : m + 1], wo.rearrange("(m p) -> p m", p=128)[:, m : m + 1]
                )

            h_sb = cpool.tile([128, W], F32, name="h_sb")
            nc.vector.memset(h_sb[:, :], 0.0)
            tanh0 = cpool.tile([128, W], mmdt, name="tanh0")
            nc.vector.memset(tanh0[:, :], 0.0)
            ident_sb = cpool.tile([128, 128], F16, name="ident_sb")
            nc.sync.dma_start(ident_sb[:, :], ident[:, :])

            # ---- U precompute: U[p, t*W + m*BL + b] ----
            # Blocked by t so the recurrence can start after the first block
            # while later blocks overlap it.
            U = upool.tile([128, T * W], F32, name="U")
            Utw = U.rearrange("p (t w) -> p t w", w=W)  # [p, t, (m,b)]
            TBLK = 128

            # ---- recurrence + interleaved U precompute ----
            # Two half-tiles (A: m=0,1 / B: m=2,3) with separate PSUM banks.
            # The leak term v = 0.9*h + U_t is accumulated into the drive PSUM
            # via an identity matmul, so the update chain is just
            # psum -> tanh -> bf16 (ScalarE reads PSUM directly; no DVE TT on
            # the critical path). h state lives only in PSUM; the next step's
            # v-STT reads the previous step's psum tile.
            HW = W // 2  # 32 cols per half
            NZB = 8  # batches per z psum bank ([1, NZB*SG] fp32 <= 512)
            with (
                tc.tile_pool(name="xt", bufs=2) as xpool,
                tc.tile_pool(name="nt", bufs=1) as npool,
                tc.tile_pool(name="ppre", bufs=2, space="PSUM") as prepool,
                tc.tile_pool(name="driveA", bufs=2, space="PSUM") as dpoolA,
                tc.tile_pool(name="driveB", bufs=2, space="PSUM") as dpoolB,
                tc.tile_pool(name="zps", bufs=2, space="PSUM") as zpool,
                tc.tile_pool(name="vtile", bufs=3) as vpool,
                tc.tile_pool(name="bstep", bufs=3) as bpool,
                tc.tile_pool(name="stg", bufs=3) as rpool,
            ):

                # U viewed [p, m, b, t] for batched (all-b) precompute ops
                Umbt = U.rearrange("p (t m b) -> p m b t", m=MT, b=BL)

                def emit_pre_block(t0b, t1b):
                    n = t1b - t0b
                    # pass A: one 1MB DMA + one big [p, b, t] TS per m
                    for m in range(MT):
                        ntile = npool.tile([128, BL * TBLK], F32, name="ntile", tag="nt")
                        ntv = ntile.rearrange("p (b t) -> p b t", t=TBLK)
                        nc.sync.dma_start(
                            ntv[:, :, 0:n],
                            nl_t[m * 128 : (m + 1) * 128, :, t0b:t1b],
                        )
                        nc.vector.tensor_scalar(
                            Umbt[:, m, :, t0b:t1b],
                            ntv[:, :, 0:n],
                            ALPHA * NOISE,
                            bh_sb[:, m : m + 1],
                            op0=MULT,
                            op1=ADD,
                        )
                    # pass B: one x DMA for all b, N=256 matmuls (2 batches)
                    xbig = xpool.tile([128, BL * TBLK], mmdt, name="xbig", tag="xt")
                    xbv = xbig.rearrange("p (b t) -> p b t", t=TBLK)
                    nc.sync.dma_start(xbv[:, :, 0:n], xl_t[:, :, t0b:t1b])
                    for m in range(MT):
                        for bb in range(0, BL, 2):
                            pps = prepool.tile([128, 2 * TBLK], F32, name="pps", tag="pps")
                            ppv = pps.rearrange("p (b t) -> p b t", t=TBLK)
                            nc.tensor.matmul(
                                ppv[:, :, 0:n],
                                wi_sb[:, m * 128 : (m + 1) * 128],
                                xbv[:, bb : bb + 2, 0:n],
                                start=True,
                                stop=True,
                            )
                            usl = Umbt[:, m, bb : bb + 2, t0b:t1b]
                            nc.vector.scalar_tensor_tensor(
                                usl, ppv[:, :, 0:n], ALPHA, usl, op0=MULT, op1=ADD
                            )

                PRE_BLOCKS = [(0, 64), (64, 128), (128, 256), (256, 384), (384, 512)]
                PRE_EMIT_AT = {8: 2, 136: 3, 264: 4}
                emit_pre_block(*PRE_BLOCKS[0])
                emit_pre_block(*PRE_BLOCKS[1])

                # z_out[0, g*T + hb*NZB*SG + b*SG + s] = z[hb*NZB+b, g*SG+s]
                z_sb = cpool.tile([1, (T // SG) * BL * SG], F32, name="z_sb")
                prevA = h_sb[:, 0:HW]   # h_{t} per half: SBUF zeros at t=0,
                prevB = h_sb[:, HW:W]   # afterwards the previous psum tiles
                prev = tanh0            # tanh(h_t) [128, W] feeding step-t MMs
                stg = None
                prev_stg = None
                zcur = [None] * (BL // NZB)
                for t in range(t_steps):
                    bi = PRE_EMIT_AT.get(t)
                    if bi is not None and bi < len(PRE_BLOCKS):
                        emit_pre_block(*PRE_BLOCKS[bi])
                    g, s = divmod(t, SG)
                    if s == 0:
                        # staging cols: (m*BL+b)*SG + s  (t-contiguous runs)
                        stg = rpool.tile([128, W * SG], mmdt, name="stg", tag="stg")
                    # v = 0.9*h + U_t  (h read straight from last step's psum)
                    v = vpool.tile([128, W], F16, name="v", tag="v")
                    nc.vector.scalar_tensor_tensor(
                        v[:, 0:HW], prevA, 1.0 - ALPHA,
                        U[:, t * W : t * W + HW], op0=MULT, op1=ADD,
                    )
                    nc.vector.scalar_tensor_tensor(
                        v[:, HW:W], prevB, 1.0 - ALPHA,
                        U[:, t * W + HW : (t + 1) * W], op0=MULT, op1=ADD,
                    )
                    dpsA = dpoolA.tile([128, HW], F32, name="dpsA", tag="dpsA")
                    dpsB = dpoolB.tile([128, HW], F32, name="dpsB", tag="dpsB")
                    bstep = bpool.tile([128, W], mmdt, name="bstep", tag="bstep")
                    for half, dps in ((0, dpsA), (1, dpsB)):
                        for mh in range(2):
                            m = half * 2 + mh
                            for k in range(KT):
                                nc.tensor.matmul(
                                    dps[:, mh * BL : (mh + 1) * BL],
                                    wh_sb[:, (k * MT + m) * 128 : (k * MT + m + 1) * 128],
                                    prev[:, k * BL : (k + 1) * BL],
                                    start=(mh == 0 and k == 0),
                                    stop=False,
                                )
                        # leak+input term folded in via identity matmul
                        nc.tensor.matmul(
                            dps[:, :],
                            ident_sb[:, :],
                            v[:, half * HW : (half + 1) * HW],
                            start=False,
                            stop=True,
                        )
                        # tanh straight from PSUM into the bf16 matmul operand
                        nc.scalar.activation(
                            bstep[:, half * HW : (half + 1) * HW], dps[:, :], TANH
                        )
                    # off-chain: stage rates for DMA (gpsimd, idle otherwise)
                    slot = stg.rearrange("p (mb s) -> p mb s", s=SG)[:, :, s]
                    nc.gpsimd.tensor_copy(slot, bstep[:, :])
                    prevA, prevB = dpsA[:, :], dpsB[:, :]
                    prev = bstep
                    # z matmuls for the PREVIOUS group, one N=128 MM per
                    # step: each fits inside the PE chain-wait bubble
                    if prev_stg is not None and s < (BL // NZB) * 16:
                        hb, rem = divmod(s, 16)
                        m, q = divmod(rem, 4)
                        if rem == 0:
                            zcur[hb] = zpool.tile(
                                [1, NZB * SG], F32, name="zp", tag="zp"
                            )
                        nc.tensor.matmul(
                            zcur[hb][0:1, q * 128 : (q + 1) * 128],
                            wo_sb[:, m : m + 1],
                            prev_stg[:, (m * BL + hb * NZB) * SG + q * 128 : (m * BL + hb * NZB) * SG + (q + 1) * 128],
                            start=(rem == 0),
                            stop=(rem == 15),
                        )
                        if rem == 15:
                            # evacuate on DVE (keeps ScalarE free for tanh)
                            nc.vector.tensor_scalar(
                                z_sb[0:1, ((g - 1) * (BL // NZB) + hb) * NZB * SG : ((g - 1) * (BL // NZB) + hb + 1) * NZB * SG],
                                zcur[hb][0:1, :],
                                float(bo_val),
                                None,
                                op0=ADD,
                            )
                    if s == SG - 1 or t == t_steps - 1:
                        ns = s + 1
                        for m in range(MT):
                            # src: [p, (b: step SG, BL), (s: 1, ns)]
                            src = stg.rearrange(
                                "p (mb s) -> p mb s", s=SG
                            )[:, m * BL : (m + 1) * BL, 0:ns]
                            dst = rates_t[
                                m * 128 : (m + 1) * 128, :, g * SG : g * SG + ns
                            ]
                            nc.sync.dma_start(dst, src)
                        prev_stg = stg

                # z for the final group
                g = t_steps // SG - 1
                for hb in range(BL // NZB):
                    zp = zpool.tile([1, NZB * SG], F32, name="zp", tag="zp")
                    for m in range(MT):
                        nc.tensor.matmul(
                            zp[0:1, :],
                            wo_sb[:, m : m + 1],
                            prev_stg[:, (m * BL + hb * NZB) * SG : (m * BL + (hb + 1) * NZB) * SG],
                            start=(m == 0),
                            stop=(m == MT - 1),
                        )
                    nc.vector.tensor_scalar(
                        z_sb[0:1, (g * (BL // NZB) + hb) * NZB * SG : (g * (BL // NZB) + hb + 1) * NZB * SG],
                        zp[0:1, :],
                        float(bo_val),
                        None,
                        op0=ADD,
                    )

                # ---- z output (grouped layout; host decodes) ----
                nc.sync.dma_start(z_o[:], z_sb[0:1, :])

    _split_excess_waits(nc)
    return nc


def _prep_inputs(x, noise, Wh, bh, Wi, Wo, mm_dtype=MM_DTYPE):
    import ml_dtypes

    mmdt_np = ml_dtypes.bfloat16 if mm_dtype == "bf16" else np.float32
    whp = (ALPHA * np.asarray(Wh, np.float32)).astype(mmdt_np)
    bhp = (ALPHA * np.asarray(bh, np.float32)).astype(np.float32)
    wi = np.ascontiguousarray(np.asarray(Wi, np.float32)).astype(mmdt_np)
    wo = np.asarray(Wo, np.float32).reshape(H).astype(mmdt_np)
    x = np.asarray(x, np.float32)
    noise = np.asarray(noise, np.float32)
    in_maps = []
    for c in range(NCORES):
        sl = slice(c * BL, (c + 1) * BL)
        in_maps.append({
            "xl_t": np.ascontiguousarray(x[sl].transpose(2, 0, 1)).astype(mmdt_np),
            "nl_t": np.ascontiguousarray(noise[sl].transpose(2, 0, 1)),
            "whp": whp,
            "wi": wi,
            "bhp": bhp,
            "wo": wo,
            "ident": np.eye(128, dtype=np.float16),
        })
    return in_maps


def _gather_outputs(results):
    rates = np.concatenate(
        [
            np.ascontiguousarray(
                r["rates_t"].astype(np.float32).transpose(1, 2, 0)
            )
            for r in results
        ],
        axis=0,
    )
    zs = []
    for r in results:
        # [g, hb, b, s] -> z[hb*NZB+b, g*SG+s]
        nhb = 4 if SG == 128 else 2
        zg = np.asarray(r["z"], np.float32).reshape(T // SG, nhb, BL // nhb, SG)
        zs.append(zg.transpose(1, 2, 0, 3).reshape(BL, T))
    z = np.concatenate(zs, axis=0)
    return z.reshape(B, T, 1).astype(np.float32), rates.astype(np.float32)


def kernel(x, noise, Wh, bh, Wi, Wo, bo):
    bo_val = float(np.asarray(bo).reshape(-1)[0])
    nc = build_nc(bo_val, MM_DTYPE)
    in_maps = _prep_inputs(x, noise, Wh, bh, Wi, Wo)
    res = run_bass_kernel_spmd(nc, in_maps, list(range(NCORES)))
    return _gather_outputs(res.results)


# revision 47
# speedup vs baseline: 1.0310x; 1.0310x over previous
"""EulerCTRNN Bass/Tile kernel for Trainium2, data-parallel over batch on 8 cores.

Math (per step t):
    h'      = 0.9*h + tanh(h) @ (0.1*Wh) + U_t
    U_t     = 0.1*(x_t @ Wi) + 0.1*bh + 0.01*noise_t   (precomputed, SBUF-resident)
    rates_t = tanh(h')
    z_t     = rates_t @ Wo + bo        (accumulated in a PSUM bank via tiny MMs)

On-chip layout is transposed: h, tanh(h), U live as [128 partitions = h%128,
cols] with cols m*BL+b (m = h//128). The recurrent matmul runs with Wh' tiles
stationary and the small batch dim moving:
    drive_T[m] = sum_k Wh'[k-tile, m-tile].T @ tanh_T[k-tile]

Host does pure data marshalling: slicing the batch across cores, transposing
x/noise to [feat, b, t] (t-contiguous) for DMA-friendly loads, and transposing
the [H, BL, T] rates output back to [B, T, H].
"""

import numpy as np

try:
    import concourse.bass as bass
except ImportError:  # concourse lives in /opt/trn_rl_repo on these images
    import sys
    sys.path.insert(0, "/opt/trn_rl_repo")
    import concourse.bass as bass

import concourse.mybir as mybir
from concourse.tile import TileContext
from concourse.vector_clock import ScopedClock
from concourse.tile_rust import add_dep_helper
from concourse.bass_utils import run_bass_kernel_spmd

F32 = mybir.dt.float32
BF16 = mybir.dt.bfloat16
F16 = mybir.dt.float16
MULT = mybir.AluOpType.mult
ADD = mybir.AluOpType.add
TANH = mybir.ActivationFunctionType.Tanh
COPY = mybir.ActivationFunctionType.Copy

# Problem constants (hardcoded per contest rules).
B, T, I, H = 128, 512, 128, 512
NCORES = 8
BL = B // NCORES          # 16 batch rows per core
KT = H // 128             # 4 contraction tiles
MT = H // 128             # 4 output tiles
W = MT * BL               # 64 = cols of one step tile
SG = 64                   # steps per rates-staging DMA group (=> 256B runs)
ALPHA = 0.1
NOISE = 0.1

# Wh matmul operand dtype: bf16 halves the LDWEIGHTS stream via FWL.
MM_DTYPE = "bf16"


def _install_drain_patch():
    """walrus here rejects >1 sync-wait on a CTRL (Drain) instruction; Tile's
    tail drain aggregates one wait per logical processor onto one Drain.
    Split into one Drain per wait (same engine, program order => equivalent)."""
    if getattr(TileContext, "_ant_drain_patch", False):
        return

    def _patched(self, tick_clock, wait_clock):
        nc = self.nc
        drain_inst = nc.sync.drain()
        wait_clock.add_sem_waits(
            drain_inst.ins, ScopedClock({None: tick_clock.global_clock})
        )
        si = drain_inst.ins.sync_info
        waits = list(si.on_wait) if si is not None else []
        if len(waits) > 1:
            si.on_wait.clear()
            si.on_wait.append(waits[0])
            for w in waits[1:]:
                extra = nc.sync.drain()
                extra.ins.sync_info = mybir.SyncInfo(on_wait=[w], on_update=[])
        nc.all_engine_barrier()
        assert self.sems is not None
        popped = nc._tile_sem_poison_stack.pop()
        assert popped is self._sem_poison
        nc.clear_and_free_semaphores(list(self.sems.allocated().values()))
        nc.all_engine_barrier()

    TileContext._drain_and_barrier = _patched
    TileContext._ant_drain_patch = True


_MAX_WAITS = 1  # this walrus build rejects >1 sync-wait per instruction


def _install_ldw_opt_patch():
    """Re-enable walrus LDWEIGHTS elision (repeated identical stationaries,
    e.g. the z-pass wo reloads). bass_utils hardcodes =false; intercept the
    walrus command line."""
    return  # =true crashes this walrus build (visitInstLdweights); keep off


def _split_excess_waits(nc):
    """Hoist overflow sync-waits onto standalone EventSemaphore instructions
    inserted just before the offending instruction on the same engine."""
    seq = 0
    for f in nc.m.functions:
        for blk in f.blocks:
            insts = blk.instructions
            i = 0
            while i < len(insts):
                inst = insts[i]
                si = getattr(inst, "sync_info", None)
                waits = list(si.on_wait) if si is not None and si.on_wait else []
                if len(waits) > _MAX_WAITS:
                    keep = waits[: _MAX_WAITS]
                    overflow = waits[_MAX_WAITS :]
                    si.on_wait.clear()
                    for w in keep:
                        si.on_wait.append(w)
                    pos = i
                    for j in range(0, len(overflow), _MAX_WAITS):
                        seq += 1
                        ev = mybir.InstEventSemaphore(
                            name=f"antwaitsplit-{seq}",
                            engine=inst.engine,
                            ins=[],
                            outs=[],
                            sync_info=mybir.SyncInfo(
                                on_wait=list(overflow[j : j + _MAX_WAITS]),
                                on_update=[],
                            ),
                        )
                        insts.insert(pos, ev)
                        pos += 1
                        i += 1
                i += 1


def build_nc(bo_val: float, mm_dtype: str = MM_DTYPE, t_steps: int = T):
    _install_drain_patch()
    _install_ldw_opt_patch()
    mmdt = BF16 if mm_dtype == "bf16" else F32
    nc = bass.Bass()

    # Host-marshalled inputs: xl_t = x[core].T -> [I, BL, T]; nl_t likewise.
    xl_t = nc.dram_tensor("xl_t", [I, BL, T], mmdt, kind="ExternalInput")
    nl_t = nc.dram_tensor("nl_t", [H, BL, T], F32, kind="ExternalInput")
    whp = nc.dram_tensor("whp", [H, H], mmdt, kind="ExternalInput")
    wi = nc.dram_tensor("wi", [I, H], mmdt, kind="ExternalInput")
    bhp = nc.dram_tensor("bhp", [H], F32, kind="ExternalInput")
    wo = nc.dram_tensor("wo", [H], mmdt, kind="ExternalInput")
    ident = nc.dram_tensor("ident", [128, 128], F16, kind="ExternalInput")
    rates_t = nc.dram_tensor("rates_t", [H, BL, T], mmdt, kind="ExternalOutput")
    # z in grouped layout: [g, hb, b, s] = z[hb*8+b, g*SG+s]; host decodes
    z_o = nc.dram_tensor("z", [(T // SG) * BL * SG], F32, kind="ExternalOutput")
    assert t_steps % SG == 0, "z group pass assumes full SG groups"

    with TileContext(nc) as tc:
        with (
            tc.tile_pool(name="const", bufs=1) as cpool,
            tc.tile_pool(name="ubig", bufs=1) as upool,
        ):
            # ---- weights / state ----
            wh_sb = cpool.tile([128, KT * MT * 128], mmdt, name="wh_sb")
            for k in range(KT):
                # wh_sb col (k*MT+m)*128+j = Wh'[k*128+p, m*128+j]
                nc.sync.dma_start(
                    wh_sb[:, k * 512 : (k + 1) * 512],
                    whp[k * 128 : (k + 1) * 128, :],
                )
            wi_sb = cpool.tile([128, MT * 128], mmdt, name="wi_sb")
            nc.sync.dma_start(wi_sb[:, :], wi[:, :])
            bh_sb = cpool.tile([128, MT], F32, name="bh_sb")
            wo_sb = cpool.tile([128, MT], mmdt, name="wo_sb")
            for m in range(MT):
                nc.sync.dma_start(
                    bh_sb[:, m : m + 1], bhp.rearrange("(m p) -> p m", p=128)[:, m : m + 1]
                )
                nc.sync.dma_start(
                    wo_sb[:, m : m + 1], wo.rearrange("(m p) -> p m", p=128)[:, m : m + 1]
                )

            h_sb = cpool.tile([128, W], F32, name="h_sb")
            nc.vector.memset(h_sb[:, :], 0.0)
            tanh0 = cpool.tile([128, W], mmdt, name="tanh0")
            nc.vector.memset(tanh0[:, :], 0.0)
            ident_sb = cpool.tile([128, 128], F16, name="ident_sb")
            nc.sync.dma_start(ident_sb[:, :], ident[:, :])

            # ---- U precompute: U[p, t*W + m*BL + b] ----
            # Blocked by t so the recurrence can start after the first block
            # while later blocks overlap it.
            U = upool.tile([128, T * W], F32, name="U")
            Utw = U.rearrange("p (t w) -> p t w", w=W)  # [p, t, (m,b)]
            TBLK = 128

            # ---- recurrence + interleaved U precompute ----
            # Two half-tiles (A: m=0,1 / B: m=2,3) with separate PSUM banks.
            # The leak term v = 0.9*h + U_t is accumulated into the drive PSUM
            # via an identity matmul, so the update chain is just
            # psum -> tanh -> bf16 (ScalarE reads PSUM directly; no DVE TT on
            # the critical path). h state lives only in PSUM; the next step's
            # v-STT reads the previous step's psum tile.
            HW = W // 2  # 32 cols per half
            NZB = 8  # batches per z psum bank ([1, NZB*SG] fp32 <= 512)
            with (
                tc.tile_pool(name="xt", bufs=2) as xpool,
                tc.tile_pool(name="nt", bufs=1) as npool,
                tc.tile_pool(name="ppre", bufs=2, space="PSUM") as prepool,
                tc.tile_pool(name="driveA", bufs=2, space="PSUM") as dpoolA,
                tc.tile_pool(name="driveB", bufs=2, space="PSUM") as dpoolB,
                tc.tile_pool(name="zps", bufs=2, space="PSUM") as zpool,
                tc.tile_pool(name="vtile", bufs=3) as vpool,
                tc.tile_pool(name="bstep", bufs=3) as bpool,
                tc.tile_pool(name="stg", bufs=3) as rpool,
            ):

                # U viewed [p, m, b, t] for batched (all-b) precompute ops
                Umbt = U.rearrange("p (t m b) -> p m b t", m=MT, b=BL)

                def emit_pre_block(t0b, t1b):
                    n = t1b - t0b
                    # pass A: one 1MB DMA + one big [p, b, t] TS per m
                    for m in range(MT):
                        ntile = npool.tile([128, BL * TBLK], F32, name="ntile", tag="nt")
                        ntv = ntile.rearrange("p (b t) -> p b t", t=TBLK)
                        nc.sync.dma_start(
                            ntv[:, :, 0:n],
                            nl_t[m * 128 : (m + 1) * 128, :, t0b:t1b],
                        )
                        for bb in range(0, BL, 4):
                            nc.vector.tensor_scalar(
                                Umbt[:, m, bb : bb + 4, t0b:t1b],
                                ntv[:, bb : bb + 4, 0:n],
                                ALPHA * NOISE,
                                bh_sb[:, m : m + 1],
                                op0=MULT,
                                op1=ADD,
                            )
                    # pass B: one x DMA for all b, N=256 matmuls (2 batches)
                    xbig = xpool.tile([128, BL * TBLK], mmdt, name="xbig", tag="xt")
                    xbv = xbig.rearrange("p (b t) -> p b t", t=TBLK)
                    nc.sync.dma_start(xbv[:, :, 0:n], xl_t[:, :, t0b:t1b])
                    for m in range(MT):
                        for bb in range(0, BL, 2):
                            pps = prepool.tile([128, 2 * TBLK], F32, name="pps", tag="pps")
                            ppv = pps.rearrange("p (b t) -> p b t", t=TBLK)
                            nc.tensor.matmul(
                                ppv[:, :, 0:n],
                                wi_sb[:, m * 128 : (m + 1) * 128],
                                xbv[:, bb : bb + 2, 0:n],
                                start=True,
                                stop=True,
                            )
                            usl = Umbt[:, m, bb : bb + 2, t0b:t1b]
                            nc.vector.scalar_tensor_tensor(
                                usl, ppv[:, :, 0:n], ALPHA, usl, op0=MULT, op1=ADD
                            )

                PRE_BLOCKS = [(0, 64), (64, 128), (128, 256), (256, 384), (384, 512)]
                PRE_EMIT_AT = {8: 2, 136: 3, 264: 4}
                emit_pre_block(*PRE_BLOCKS[0])
                emit_pre_block(*PRE_BLOCKS[1])

                # z_out[0, g*T + hb*NZB*SG + b*SG + s] = z[hb*NZB+b, g*SG+s]
                z_sb = cpool.tile([1, (T // SG) * BL * SG], F32, name="z_sb")
                prevA = h_sb[:, 0:HW]   # h_{t} per half: SBUF zeros at t=0,
                prevB = h_sb[:, HW:W]   # afterwards the previous psum tiles
                prev = tanh0            # tanh(h_t) [128, W] feeding step-t MMs
                stg = None
                prev_stg = None
                zcur = [None] * (BL // NZB)
                for t in range(t_steps):
                    bi = PRE_EMIT_AT.get(t)
                    if bi is not None and bi < len(PRE_BLOCKS):
                        emit_pre_block(*PRE_BLOCKS[bi])
                    g, s = divmod(t, SG)
                    if s == 0:
                        # staging cols: (m*BL+b)*SG + s  (t-contiguous runs)
                        stg = rpool.tile([128, W * SG], mmdt, name="stg", tag="stg")
                    # v = 0.9*h + U_t  (h read straight from last step's psum)
                    v = vpool.tile([128, W], F16, name="v", tag="v")
                    nc.vector.scalar_tensor_tensor(
                        v[:, 0:HW], prevA, 1.0 - ALPHA,
                        U[:, t * W : t * W + HW], op0=MULT, op1=ADD,
                    )
                    nc.vector.scalar_tensor_tensor(
                        v[:, HW:W], prevB, 1.0 - ALPHA,
                        U[:, t * W + HW : (t + 1) * W], op0=MULT, op1=ADD,
                    )
                    dpsA = dpoolA.tile([128, HW], F32, name="dpsA", tag="dpsA")
                    dpsB = dpoolB.tile([128, HW], F32, name="dpsB", tag="dpsB")
                    bstep = bpool.tile([128, W], mmdt, name="bstep", tag="bstep")
                    for half, dps in ((0, dpsA), (1, dpsB)):
                        for mh in range(2):
                            m = half * 2 + mh
                            for k in range(KT):
                                nc.tensor.matmul(
                                    dps[:, mh * BL : (mh + 1) * BL],
                                    wh_sb[:, (k * MT + m) * 128 : (k * MT + m + 1) * 128],
                                    prev[:, k * BL : (k + 1) * BL],
                                    start=(mh == 0 and k == 0),
                                    stop=False,
                                )
                        # leak+input term folded in via identity matmul
                        last_mm = nc.tensor.matmul(
                            dps[:, :],
                            ident_sb[:, :],
                            v[:, half * HW : (half + 1) * HW],
                            start=False,
                            stop=True,
                        )
                        # tanh straight from PSUM into the bf16 matmul operand
                        nc.scalar.activation(
                            bstep[:, half * HW : (half + 1) * HW], dps[:, :], TANH
                        )
                    # off-chain: stage rates for DMA (gpsimd, idle otherwise)
                    slot = stg.rearrange("p (mb s) -> p mb s", s=SG)[:, :, s]
                    nc.gpsimd.tensor_copy(slot, bstep[:, :])
                    prevA, prevB = dpsA[:, :], dpsB[:, :]
                    prev = bstep
                    # z matmuls for the PREVIOUS group, one N=128 MM per
                    # step: each fits inside the PE chain-wait bubble
                    if prev_stg is not None and s < (BL // NZB) * 16:
                        hb, rem = divmod(s, 16)
                        m, q = divmod(rem, 4)
                        if rem == 0:
                            zcur[hb] = zpool.tile(
                                [1, NZB * SG], F32, name="zp", tag="zp"
                            )
                        zmm = nc.tensor.matmul(
                            zcur[hb][0:1, q * 128 : (q + 1) * 128],
                            wo_sb[:, m : m + 1],
                            prev_stg[:, (m * BL + hb * NZB) * SG + q * 128 : (m * BL + hb * NZB) * SG + (q + 1) * 128],
                            start=(rem == 0),
                            stop=(rem == 15),
                        )
                        add_dep_helper(
                            zmm.ins, last_mm.ins, sync=False,
                            reason="z filler after drive MMs",
                        )
                        if rem == 15:
                            # evacuate on DVE (keeps ScalarE free for tanh)
                            nc.vector.tensor_scalar(
                                z_sb[0:1, ((g - 1) * (BL // NZB) + hb) * NZB * SG : ((g - 1) * (BL // NZB) + hb + 1) * NZB * SG],
                                zcur[hb][0:1, :],
                                float(bo_val),
                                None,
                                op0=ADD,
                            )
                    if s == SG - 1 or t == t_steps - 1:
                        ns = s + 1
                        for m in range(MT):
                            # src: [p, (b: step SG, BL), (s: 1, ns)]
                            src = stg.rearrange(
                                "p (mb s) -> p mb s", s=SG
                            )[:, m * BL : (m + 1) * BL, 0:ns]
                            dst = rates_t[
                                m * 128 : (m + 1) * 128, :, g * SG : g * SG + ns
                            ]
                            nc.sync.dma_start(dst, src)
                        prev_stg = stg

                # z for the final group
                g = t_steps // SG - 1
                for hb in range(BL // NZB):
                    zp = zpool.tile([1, NZB * SG], F32, name="zp", tag="zp")
                    for m in range(MT):
                        nc.tensor.matmul(
                            zp[0:1, :],
                            wo_sb[:, m : m + 1],
                            prev_stg[:, (m * BL + hb * NZB) * SG : (m * BL + (hb + 1) * NZB) * SG],
                            start=(m == 0),
                            stop=(m == MT - 1),
                        )
                    nc.vector.tensor_scalar(
                        z_sb[0:1, (g * (BL // NZB) + hb) * NZB * SG : (g * (BL // NZB) + hb + 1) * NZB * SG],
                        zp[0:1, :],
                        float(bo_val),
                        None,
                        op0=ADD,
                    )

                # ---- z output (grouped layout; host decodes) ----
                nc.sync.dma_start(z_o[:], z_sb[0:1, :])

    _split_excess_waits(nc)
    return nc


def _prep_inputs(x, noise, Wh, bh, Wi, Wo, mm_dtype=MM_DTYPE):
    import ml_dtypes

    mmdt_np = ml_dtypes.bfloat16 if mm_dtype == "bf16" else np.float32
    whp = (ALPHA * np.asarray(Wh, np.float32)).astype(mmdt_np)
    bhp = (ALPHA * np.asarray(bh, np.float32)).astype(np.float32)
    wi = np.ascontiguousarray(np.asarray(Wi, np.float32)).astype(mmdt_np)
    wo = np.asarray(Wo, np.float32).reshape(H).astype(mmdt_np)
    x = np.asarray(x, np.float32)
    noise = np.asarray(noise, np.float32)
    in_maps = []
    for c in range(NCORES):
        sl = slice(c * BL, (c + 1) * BL)
        in_maps.append({
            "xl_t": np.ascontiguousarray(x[sl].transpose(2, 0, 1)).astype(mmdt_np),
            "nl_t": np.ascontiguousarray(noise[sl].transpose(2, 0, 1)),
            "whp": whp,
            "wi": wi,
            "bhp": bhp,
            "wo": wo,
            "ident": np.eye(128, dtype=np.float16),
        })
    return in_maps


def _gather_outputs(results):
    rates = np.concatenate(
        [
            np.ascontiguousarray(
                r["rates_t"].astype(np.float32).transpose(1, 2, 0)
            )
            for r in results
        ],
        axis=0,
    )
    zs = []
    for r in results:
        # [g, hb, b, s] -> z[hb*NZB+b, g*SG+s]
        nhb = 4 if SG == 128 else 2
        zg = np.asarray(r["z"], np.float32).reshape(T // SG, nhb, BL // nhb, SG)
        zs.append(zg.transpose(1, 2, 0, 3).reshape(BL, T))
    z = np.concatenate(zs, axis=0)
    return z.reshape(B, T, 1).astype(np.float32), rates.astype(np.float32)


def kernel(x, noise, Wh, bh, Wi, Wo, bo):
    bo_val = float(np.asarray(bo).reshape(-1)[0])
    nc = build_nc(bo_val, MM_DTYPE)
    in_maps = _prep_inputs(x, noise, Wh, bh, Wi, Wo)
    res = run_bass_kernel_spmd(nc, in_maps, list(range(NCORES)))
    return _gather_outputs(res.results)
